# revision 43
# baseline (speedup 1.0000x reference)
"""Trainium2 Bass kernel for GQA attention (B=2, L=2048, D=2048, H=16, KV=8, HD=128).

Sharding: tensor-parallel over heads across 8 cores (2 Q heads + 1 KV head per
core), flash-style attention per core, then two AllToAlls (one per local head)
to redistribute from head-sharding to token-sharding before the output
projection (each core computes 512 full output rows; host concatenates).

All matmuls run as fp32r (full fp32 storage, 1 cycle/row at N>=512).
1/sqrt and 1/x are computed via the ACT Dsqrt table (0.5*x^-0.5) broadcast
through a K=1 matmul with a 2.0 stationary vector.
"""
import math
import numpy as np

B, L, D = 2, 2048, 2048
H, KV, HD = 16, 8, 128
NCORES = 8
T = B * L            # 4096 tokens, b-major
TPC = T // NCORES    # 512 tokens per core after A2A
HPC = H // NCORES    # 2 local query heads
EPS = 1e-5
ROPE_BASE = 10000.0
SCALE = HD ** -0.5

TT = 512             # token tile (free dim)
NTT = L // TT        # 4 token tiles per batch
NDC = D // 128       # 16 contraction chunks
NFC = 4              # output col chunks of 128 in qkv proj (2 q heads + k + v)

_CACHE = {}
DEBUG = False


def _rope_tables():
    """cos/sin LUTs [64, L] computed exactly like the jax reference (f32, cpu)."""
    import jax
    import jax.numpy as jnp

    cpu = jax.devices("cpu")[0]
    with jax.default_device(cpu):
        base = ROPE_BASE * 1.0 ** (HD / (HD - 2))
        freqs = base ** (jnp.arange(0, HD, 2, dtype=jnp.float32) / HD)   # [64]
        pos = jnp.arange(L, dtype=jnp.float32)                           # [L]
        angles = pos[:, None] * freqs[None, :]                           # [L, 64]
        cos = np.asarray(jnp.cos(angles), dtype=np.float32).T.copy()     # [64, L]
        sin = np.asarray(jnp.sin(angles), dtype=np.float32).T.copy()
    return cos, sin


def _build_nc():
    import concourse.bass as bass
    import concourse.tile as tile
    from concourse.tile import add_dep_helper
    import concourse.mybir as mybir
    from concourse import bacc
    from concourse.masks import make_identity
    from contextlib import ExitStack

    f32 = mybir.dt.float32
    f16 = mybir.dt.float16
    Exp = mybir.ActivationFunctionType.Exp
    Ln = mybir.ActivationFunctionType.Ln
    mult = mybir.AluOpType.mult
    add = mybir.AluOpType.add
    sub = mybir.AluOpType.subtract

    from concourse import bacc as _bacc_mod
    from concourse import hw_specs as _hw

    if not getattr(_bacc_mod, "_act_table_patch", False):
        _orig_get = _bacc_mod.get_activation_tables

        def _patched_get(arch):
            t = _orig_get(arch)
            exp = mybir.ActivationFunctionType.Exp
            ln = mybir.ActivationFunctionType.Ln
            for name, funcs in t.items():
                if name != "natural_log_exp_and_others":
                    funcs.discard(exp)
                    funcs.discard(ln)
            return t

        _bacc_mod.get_activation_tables = _patched_get
        _bacc_mod._act_table_patch = True

    nc = bacc.Bacc(num_devices=NCORES)

    xT = nc.dram_tensor("xT", [D, T], f16, kind="ExternalInput")
    wqkv = nc.dram_tensor("wqkv", [D, 512], f16, kind="ExternalInput")
    wo = nc.dram_tensor("wo", [D, D], f16, kind="ExternalInput")
    lcos = nc.dram_tensor("lcos", [64, L], f16, kind="ExternalInput")
    lsin = nc.dram_tensor("lsin", [64, L], f16, kind="ExternalInput")
    qn = nc.dram_tensor("qn", [HD, 1], f32, kind="ExternalInput")
    kn = nc.dram_tensor("kn", [HD, 1], f32, kind="ExternalInput")
    yT = nc.dram_tensor("yT", [D, TPC], f32, kind="ExternalOutput")

    # A2A bounce buffers, one pair per local head chunk
    cc_in = [nc.dram_tensor(f"cc_in{h}", [NCORES, HD, TPC], f16) for h in range(HPC)]
    cc_out = [nc.dram_tensor(f"cc_out{h}", [NCORES, HD, TPC], f16) for h in range(HPC)]

    if DEBUG:
        dbg_q = nc.dram_tensor("dbg_q", [128, HPC, T], f16, kind="ExternalOutput")
        dbg_k = nc.dram_tensor("dbg_k", [128, T], f16, kind="ExternalOutput")
        dbg_v = nc.dram_tensor("dbg_v", [128, T // 128, HD], f16, kind="ExternalOutput")
        dbg_ag = nc.dram_tensor("dbg_ag", [128, NDC, TPC], f16, kind="ExternalOutput")

    with tile.TileContext(nc) as tc, ExitStack() as ctx, nc.allow_low_precision(
        reason="f16 tiles are full fp32 storage; all accumulation is fp32 PSUM"
    ):
        consts = ctx.enter_context(tc.tile_pool(name="consts", bufs=1))
        xtp = ctx.enter_context(tc.tile_pool(name="xtp", bufs=18))
        qkvp = ctx.enter_context(tc.tile_pool(name="qkvp", bufs=1))
        ropep = ctx.enter_context(tc.tile_pool(name="ropep", bufs=3))
        halfp = ctx.enter_context(tc.tile_pool(name="halfp", bufs=6))
        statp = ctx.enter_context(tc.tile_pool(name="statp", bufs=4))
        sap = ctx.enter_context(tc.tile_pool(name="sap", bufs=6))
        expp = ctx.enter_context(tc.tile_pool(name="expp", bufs=8))
        attp = ctx.enter_context(tc.tile_pool(name="attp", bufs=4))
        wop = ctx.enter_context(tc.tile_pool(name="wop", bufs=6))
        wop2 = ctx.enter_context(tc.tile_pool(name="wop2", bufs=6))
        yp = ctx.enter_context(tc.tile_pool(name="yp", bufs=2))
        yep = ctx.enter_context(tc.tile_pool(name="yep", bufs=16))

        pacc = ctx.enter_context(tc.tile_pool(name="pacc", bufs=2, space="PSUM"))
        pstream = ctx.enter_context(tc.tile_pool(name="pstream", bufs=5, space="PSUM"))
        pmisc = ctx.enter_context(tc.tile_pool(name="pmisc", bufs=1, space="PSUM"))

        # ---- constants ----
        ones_f = consts.tile([128, 1], f32)
        nc.vector.memset(ones_f, 1.0)
        ones = consts.tile([128, 1], f16)
        nc.vector.tensor_copy(out=ones, in_=ones_f)
        ones_k1_f = consts.tile([1, 128], f32)
        nc.vector.memset(ones_k1_f, 1.0)
        ones_k1 = consts.tile([1, 128], f16)
        nc.vector.tensor_copy(out=ones_k1, in_=ones_k1_f)
        ident = consts.tile([128, 128], f16)
        make_identity(nc, ident)
        eps_t = consts.tile([1, 1], f32)
        nc.vector.memset(eps_t, EPS)
        # LUTs duplicated into both partition halves so rope tensor_tensor
        # ops always see matching base partitions
        cos_sb = consts.tile([128, L], f16)
        nc.gpsimd.dma_start(out=cos_sb[0:64, :], in_=lcos[:, :])
        nc.gpsimd.dma_start(out=cos_sb[64:128, :], in_=lcos[:, :])
        sin_sb = consts.tile([128, L], f16)
        nc.gpsimd.dma_start(out=sin_sb[0:64, :], in_=lsin[:, :])
        nc.gpsimd.dma_start(out=sin_sb[64:128, :], in_=lsin[:, :])
        qn_sb = consts.tile([HD, 1], f32)
        nc.gpsimd.dma_start(out=qn_sb, in_=qn[:, :])
        kn_sb = consts.tile([HD, 1], f32)
        nc.gpsimd.dma_start(out=kn_sb, in_=kn[:, :])

        # ---- persistent activations, per batch ----
        qh_t = [
            [
                qkvp.tile([128, L], f16, tag=f"ag{h}", bufs=2, name=f"qh{h}{b}")
                for b in range(B)
            ]
            for h in range(HPC)
        ]
        kh_t = [
            qkvp.tile([128, L], f16, tag=f"kh{b}", name=f"kh{b}") for b in range(B)
        ]
        v_t = [
            qkvp.tile([128, L // 128, HD], f16, tag=f"v{b}", name=f"v{b}")
            for b in range(B)
        ]

        # ---- weights for qkv projection (resident) ----
        w_sb = consts.tile([128, NDC, 512], f16)
        _wr = wqkv.ap().rearrange("(dc p) f -> p dc f", p=128)
        nc.sync.dma_start(out=w_sb[:, 0:4, :], in_=_wr[:, 0:4, :])
        nc.sync.dma_start(out=w_sb[:, 4:NDC, :], in_=_wr[:, 4:NDC, :])

        def proj_tt(b, tt):
            """QKV projection + RoPE + RMSNorm for one 512-token tile.

            Uses only 2 PSUM slots (two fc passes over resident x tiles) so a
            concurrent attention stream can hold the other two."""
            pos0 = tt * TT
            tok0 = b * L + tt * TT
            xts = []
            dma_eng = nc.scalar if (b == 0 and tt == 0) else nc.sync
            for dc in range(NDC):
                xt = xtp.tile([128, TT], f16, tag="xt")
                dma_eng.dma_start(
                    out=xt,
                    in_=xT[dc * 128:(dc + 1) * 128, tok0:tok0 + TT],
                )
                xts.append(xt)
            for fc in range(NFC):
                pp = pacc.tile([128, TT], f32, tag="pacc", name=f"pp{fc}")
                for dc in range(NDC):
                    nc.tensor.matmul(
                        pp,
                        w_sb[:, dc, fc * 128:(fc + 1) * 128],
                        xts[dc],
                        start=(dc == 0),
                        stop=(dc == NDC - 1),
                    )
                if True:
                    if fc < 3:
                        # copy psum to sbuf once (frees psum), rope on fp16
                        rsrc = ropep.tile([128, TT], f16, tag="rsrc")
                        nc.scalar.copy(out=rsrc, in_=pp)
                        cs_lo = cos_sb[0:64, pos0:pos0 + TT]
                        cs_hi = cos_sb[64:128, pos0:pos0 + TT]
                        sn_lo = sin_sb[0:64, pos0:pos0 + TT]
                        sn_hi = sin_sb[64:128, pos0:pos0 + TT]
                        x1 = rsrc[0:64, :]
                        x2 = rsrc[64:128, :]
                        t1 = halfp.tile([64, TT], f16, tag="half")
                        t2 = halfp.tile([64, TT], f16, tag="half")
                        t3 = halfp.tile([64, TT], f16, tag="half")
                        t4 = halfp.tile([64, TT], f16, tag="half")
                        roped = ropep.tile([128, TT], f16, tag="roped")
                        nc.vector.tensor_tensor(out=t1, in0=x1, in1=cs_lo, op=mult)
                        nc.vector.tensor_tensor(out=t2, in0=x2, in1=sn_hi, op=mult)
                        nc.vector.tensor_tensor(
                            out=roped[0:64, :], in0=t1, in1=t2, op=sub
                        )
                        nc.vector.tensor_tensor(out=t3, in0=x2, in1=cs_hi, op=mult)
                        nc.vector.tensor_tensor(out=t4, in0=x1, in1=sn_lo, op=mult)
                        nc.vector.tensor_tensor(
                            out=roped[64:128, :], in0=t3, in1=t4, op=add
                        )
                        # sum of squares over HD (partition) via ones-matmul
                        sq = ropep.tile([128, TT], f16, tag="sq")
                        nc.vector.tensor_tensor(out=sq, in0=roped, in1=roped, op=mult)
                        pss = pstream.tile([1, TT], f32, tag="pstream")
                        nc.tensor.matmul(pss, ones, sq, start=True, stop=True)
                        # rstd = exp(-0.5*ln(ss/HD + eps)) -- Ln/Exp share one table
                        lnt = statp.tile([1, TT], f32, tag="stat")
                        nc.scalar.activation(
                            out=lnt, in_=pss, func=Ln, bias=eps_t, scale=1.0 / HD
                        )
                        srd = statp.tile([1, TT], f16, tag="stat")
                        nc.scalar.activation(out=srd, in_=lnt, func=Exp, scale=-0.5)
                        # broadcast rstd over partitions via K=1 matmul
                        pb = pmisc.tile([128, TT], f32, tag="pmisc")
                        nc.tensor.matmul(pb, ones_k1, srd, start=True, stop=True)
                        # final: out = (roped * norm_w) * rstd_bcast
                        w_head = qn_sb if fc < 2 else kn_sb
                        if fc < 2:
                            dst = qh_t[fc][b][:, pos0:pos0 + TT]
                        else:
                            dst = kh_t[b][:, pos0:pos0 + TT]
                        nc.vector.scalar_tensor_tensor(
                            out=dst, in0=roped, scalar=w_head, in1=pb,
                            op0=mult, op1=mult,
                        )
                    else:
                        # v: copy out and transpose to [tok, HD]
                        vt = ropep.tile([128, TT], f16, tag="rsrc")
                        nc.scalar.copy(out=vt, in_=pp)
                        for i in range(TT // 128):
                            pt = pstream.tile([128, 128], f16, tag="pstream")
                            nc.tensor.transpose(
                                pt, vt[:, i * 128:(i + 1) * 128], ident
                            )
                            nc.scalar.copy(out=v_t[b][:, tt * 4 + i, :], in_=pt)

        def att_tiles(jobs):
            """Interleaved attention for a list of (hc, b, tqt) query tiles.

            The softmax denominator is accumulated on the vector engine
            (sacc += exp tile) so the PE only runs 2 matmuls per key chunk;
            one partition-sum matmul per query tile closes it out."""
            NK = L // 128
            state = []
            for hc, b, tqt in jobs:
                qs = qh_t[hc][b][:, tqt * TT:(tqt + 1) * TT]
                po = pacc.tile([128, TT], f32, tag="pacc", name=f"po{hc}{b}{tqt}")
                sacc = [
                    sap.tile([128, TT], f16, tag="sacc", name=f"sa{p}{hc}{b}{tqt}")
                    for p in range(2)
                ]
                state.append((hc, b, tqt, qs, po, sacc))
            for tk in range(NK):
                ets = []
                for hc, b, tqt, qs, po, sacc in state:
                    ps = pstream.tile([128, TT], f32, tag="pstream")
                    nc.tensor.matmul(
                        ps, kh_t[b][:, tk * 128:(tk + 1) * 128], qs,
                        start=True, stop=True,
                    )
                    et = expp.tile([128, TT], f16, tag="expt")
                    nc.scalar.activation(out=et, in_=ps, func=Exp, scale=SCALE)
                    ets.append(et)
                for (hc, b, tqt, qs, po, sacc), et in zip(state, ets):
                    nc.tensor.matmul(
                        po, v_t[b][:, tk, :], et,
                        start=(tk == 0), stop=(tk == NK - 1),
                    )
                    sa = sacc[tk % 2]
                    if tk < 2:
                        nc.vector.tensor_copy(out=sa, in_=et)
                    else:
                        nc.vector.tensor_tensor(out=sa, in0=sa, in1=et, op=add)
            for hc, b, tqt, qs, po, sacc in state:
                # denominator: partition-sum of both fp16 partials, 1/x, bcast
                pd = pmisc.tile([1, TT], f32, tag="pmisc")
                nc.tensor.matmul(pd, ones, sacc[0], start=True, stop=False)
                nc.tensor.matmul(pd, ones, sacc[1], start=False, stop=True)
                rdf = statp.tile([1, TT], f32, tag="stat")
                nc.vector.reciprocal_approx_fast(out=rdf, in_=pd)
                rd = statp.tile([1, TT], f16, tag="stat")
                nc.vector.tensor_copy(out=rd, in_=rdf)
                pb = pmisc.tile([128, TT], f32, tag="pmisc")
                nc.tensor.matmul(pb, ones_k1, rd, start=True, stop=True)
                o_sb = attp.tile([128, TT], f32, tag="att")
                nc.vector.tensor_copy(out=o_sb, in_=po)
                aout = attp.tile([128, TT], f16, tag="att")
                nc.vector.tensor_tensor(out=aout, in0=o_sb, in1=pb, op=mult)
                j = b * NTT + tqt
                nc.sync.dma_start(out=cc_in[hc][j, :, :], in_=aout)

        def att_tile(hc, b, tqt):
            att_tiles([(hc, b, tqt)])

        # ---- phase schedule ----
        for tt in range(NTT):
            proj_tt(0, tt)
        # att(0,0) interleaved with proj(1): 2+2 PSUM slots
        for tqt in range(NTT):
            att_tile(0, 0, tqt)
            proj_tt(1, tqt)
        att_tiles([(0, 1, 0), (0, 1, 1)])
        att_tiles([(0, 1, 2), (0, 1, 3)])
        att_tiles([(1, 0, 0), (1, 0, 1)])
        att_tiles([(1, 0, 2), (1, 0, 3)])

        nc.gpsimd.collective_compute(
            "AllToAll",
            mybir.AluOpType.bypass,
            replica_groups=[list(range(NCORES))],
            ins=[cc_in[0].ap()],
            outs=[cc_out[0].ap()],
        )
        ag0a = qkvp.tile([128, 4, TPC], f16, tag="ag0", bufs=2, name="ag0a")
        ag0b = qkvp.tile([128, 4, TPC], f16, tag="ag0", bufs=2, name="ag0b")
        nc.sync.dma_start(
            out=ag0a,
            in_=cc_out[0].ap()[0:4, :, :].rearrange("j p t -> p j t"),
        )
        nc.sync.dma_start(
            out=ag0b,
            in_=cc_out[0].ap()[4:8, :, :].rearrange("j p t -> p j t"),
        )

        # out-projection is split: the even f-chunks (from ag0) accumulate
        # into SBUF partials during att(1,1) and the second AllToAll; the odd
        # half finishes after ag1 arrives.
        wos_e = {}
        wos_o = {}

        def load_wos(dc, parity, engine):
            store = wos_e if parity == 0 else wos_o
            pool = wop if parity == 0 else wop2
            store[dc] = pool.tile(
                [128, NDC // 2, 128], f16, tag=f"wo{parity}", name=f"wos{parity}_{dc}"
            )
            engine.dma_start(
                out=store[dc],
                in_=wo[:, dc * 128:(dc + 1) * 128].rearrange(
                    "(fc p) m -> p fc m", p=128
                )[:, parity::2, :],
            )

        ye_t = {}

        def even_pass(dc):
            py = pacc.tile([128, TPC], f32, tag="pacc", name="pye")
            for j in range(NDC // 2):
                srct = ag0a if j < 4 else ag0b
                nc.tensor.matmul(
                    py, wos_e[dc][:, j, :], srct[:, j % 4, :],
                    start=(j == 0), stop=(j == 7),
                )
            ye = yep.tile([128, TPC], f16, tag="ye", name=f"ye{dc}")
            nc.vector.tensor_copy(out=ye, in_=py)
            ye_t[dc] = ye

        # att(1,1) first (so the second AllToAll can fire early), then the
        # even half of the out-projection overlaps that AllToAll
        att_tiles([(1, 1, 0), (1, 1, 1)])
        att_tiles([(1, 1, 2), (1, 1, 3)])
        for dc in range(NDC):
            load_wos(dc, 0, nc.sync)
            even_pass(dc)

        nc.gpsimd.collective_compute(
            "AllToAll",
            mybir.AluOpType.bypass,
            replica_groups=[list(range(NCORES))],
            ins=[cc_in[1].ap()],
            outs=[cc_out[1].ap()],
        )
        ag1a = qkvp.tile([128, 4, TPC], f16, tag="ag1", bufs=2, name="ag1a")
        ag1b = qkvp.tile([128, 4, TPC], f16, tag="ag1", bufs=2, name="ag1b")
        nc.sync.dma_start(
            out=ag1a,
            in_=cc_out[1].ap()[0:4, :, :].rearrange("j p t -> p j t"),
        )
        nc.sync.dma_start(
            out=ag1b,
            in_=cc_out[1].ap()[4:8, :, :].rearrange("j p t -> p j t"),
        )

        if DEBUG:
            for h in range(HPC):
                for b in range(B):
                    nc.sync.dma_start(
                        out=dbg_q.ap()[:, h, b * L:(b + 1) * L],
                        in_=qh_t[h][b],
                    )
            for b in range(B):
                nc.sync.dma_start(
                    out=dbg_k.ap()[:, b * L:(b + 1) * L], in_=kh_t[b]
                )
                nc.sync.dma_start(
                    out=dbg_v.ap()[:, b * 16:(b + 1) * 16, :], in_=v_t[b]
                )
            for j in range(NCORES):
                a0 = ag0a if j < 4 else ag0b
                a1 = ag1a if j < 4 else ag1b
                nc.sync.dma_start(out=dbg_ag.ap()[:, 2 * j, :], in_=a0[:, j % 4, :])
                nc.sync.dma_start(out=dbg_ag.ap()[:, 2 * j + 1, :], in_=a1[:, j % 4, :])

        # odd half + combine + store; first slices prefetch during the A2A
        for dc in range(6):
            load_wos(dc, 1, nc.sync)
        for dc in range(NDC):
            if dc not in wos_o:
                load_wos(dc, 1, nc.sync)
            py = pacc.tile([128, TPC], f32, tag="pacc", name="pyo")
            for j in range(NDC // 2):
                srct = ag1a if j < 4 else ag1b
                nc.tensor.matmul(
                    py, wos_o[dc][:, j, :], srct[:, j % 4, :],
                    start=(j == 0), stop=(j == 7),
                )
            yt = yp.tile([128, TPC], f32, tag="y")
            nc.vector.tensor_tensor(out=yt, in0=py, in1=ye_t[dc], op=add)
            nc.sync.dma_start(out=yT[dc * 128:(dc + 1) * 128, :], in_=yt)

    nc.finalize()
    return nc


def kernel(x, wq, wk, wv, wo, qn_w, kn_w):
    from concourse.bass_utils import run_bass_kernel_spmd

    if "nc" not in _CACHE:
        _CACHE["nc"] = _build_nc()
    nc = _CACHE["nc"]

    x = np.asarray(x, dtype=np.float32)
    wq = np.asarray(wq, dtype=np.float32)
    wk = np.asarray(wk, dtype=np.float32)
    wv = np.asarray(wv, dtype=np.float32)
    wo = np.asarray(wo, dtype=np.float32)
    qn_w = np.asarray(qn_w, dtype=np.float32).reshape(HD, 1).copy()
    kn_w = np.asarray(kn_w, dtype=np.float32).reshape(HD, 1).copy()

    xT = np.ascontiguousarray(x.reshape(T, D).T.astype(np.float16))
    wo16 = wo.astype(np.float16)
    cos, sin = _rope_tables()
    cos = cos.astype(np.float16)
    sin = sin.astype(np.float16)

    in_maps = []
    for c in range(NCORES):
        wqkv_c = np.ascontiguousarray(
            np.concatenate(
                [
                    wq[:, c * HPC * HD:(c + 1) * HPC * HD],
                    wk[:, c * HD:(c + 1) * HD],
                    wv[:, c * HD:(c + 1) * HD],
                ],
                axis=1,
            ).astype(np.float16)
        )
        in_maps.append(
            {
                "xT": xT,
                "wqkv": wqkv_c,
                "wo": wo16,
                "lcos": cos,
                "lsin": sin,
                "qn": qn_w,
                "kn": kn_w,
            }
        )

    trace = bool(_CACHE.get("trace"))
    r = run_bass_kernel_spmd(
        nc, in_maps, core_ids=list(range(NCORES)), trace=trace
    )
    _CACHE["last_result"] = r

    y = np.empty((T, D), dtype=np.float32)
    for c in range(NCORES):
        y[c * TPC:(c + 1) * TPC, :] = r.results[c]["yT"].T
    return y.reshape(B, L, D)


# revision 44
# speedup vs baseline: 1.0690x; 1.0690x over previous
"""Trainium2 Bass kernel for GQA attention (B=2, L=2048, D=2048, H=16, KV=8, HD=128).

Sharding: tensor-parallel over heads across 8 cores (2 Q heads + 1 KV head per
core), flash-style attention per core, then two AllToAlls (one per local head)
to redistribute from head-sharding to token-sharding before the output
projection (each core computes 512 full output rows; host concatenates).

All matmuls run as fp32r (full fp32 storage, 1 cycle/row at N>=512).
1/sqrt and 1/x are computed via the ACT Dsqrt table (0.5*x^-0.5) broadcast
through a K=1 matmul with a 2.0 stationary vector.
"""
import math
import numpy as np

B, L, D = 2, 2048, 2048
H, KV, HD = 16, 8, 128
NCORES = 8
T = B * L            # 4096 tokens, b-major
TPC = T // NCORES    # 512 tokens per core after A2A
HPC = H // NCORES    # 2 local query heads
EPS = 1e-5
ROPE_BASE = 10000.0
SCALE = HD ** -0.5

TT = 512             # token tile (free dim)
NTT = L // TT        # 4 token tiles per batch
NDC = D // 128       # 16 contraction chunks
NFC = 4              # output col chunks of 128 in qkv proj (2 q heads + k + v)

_CACHE = {}
DEBUG = False


def _rope_tables():
    """cos/sin LUTs [64, L] computed exactly like the jax reference (f32, cpu)."""
    import jax
    import jax.numpy as jnp

    cpu = jax.devices("cpu")[0]
    with jax.default_device(cpu):
        base = ROPE_BASE * 1.0 ** (HD / (HD - 2))
        freqs = base ** (jnp.arange(0, HD, 2, dtype=jnp.float32) / HD)   # [64]
        pos = jnp.arange(L, dtype=jnp.float32)                           # [L]
        angles = pos[:, None] * freqs[None, :]                           # [L, 64]
        cos = np.asarray(jnp.cos(angles), dtype=np.float32).T.copy()     # [64, L]
        sin = np.asarray(jnp.sin(angles), dtype=np.float32).T.copy()
    return cos, sin


def _build_nc():
    import concourse.bass as bass
    import concourse.tile as tile
    from concourse.tile import add_dep_helper
    import concourse.mybir as mybir
    from concourse import bacc
    from concourse.masks import make_identity
    from contextlib import ExitStack

    f32 = mybir.dt.float32
    f16 = mybir.dt.float16
    Exp = mybir.ActivationFunctionType.Exp
    Ln = mybir.ActivationFunctionType.Ln
    mult = mybir.AluOpType.mult
    add = mybir.AluOpType.add
    sub = mybir.AluOpType.subtract

    from concourse import bacc as _bacc_mod
    from concourse import hw_specs as _hw

    if not getattr(_bacc_mod, "_act_table_patch", False):
        _orig_get = _bacc_mod.get_activation_tables

        def _patched_get(arch):
            t = _orig_get(arch)
            exp = mybir.ActivationFunctionType.Exp
            ln = mybir.ActivationFunctionType.Ln
            for name, funcs in t.items():
                if name != "natural_log_exp_and_others":
                    funcs.discard(exp)
                    funcs.discard(ln)
            return t

        _bacc_mod.get_activation_tables = _patched_get
        _bacc_mod._act_table_patch = True

    nc = bacc.Bacc(num_devices=NCORES)

    xT = nc.dram_tensor("xT", [D, T], f16, kind="ExternalInput")
    wqkv = nc.dram_tensor("wqkv", [D, 512], f16, kind="ExternalInput")
    wo = nc.dram_tensor("wo", [D, D], f16, kind="ExternalInput")
    lcos = nc.dram_tensor("lcos", [64, L], f16, kind="ExternalInput")
    lsin = nc.dram_tensor("lsin", [64, L], f16, kind="ExternalInput")
    qn = nc.dram_tensor("qn", [HD, 1], f32, kind="ExternalInput")
    kn = nc.dram_tensor("kn", [HD, 1], f32, kind="ExternalInput")
    yT = nc.dram_tensor("yT", [D, TPC], f32, kind="ExternalOutput")

    # A2A bounce buffers, one pair per local head chunk
    cc_in = [nc.dram_tensor(f"cc_in{h}", [NCORES, HD, TPC], f16) for h in range(HPC)]
    cc_out = [nc.dram_tensor(f"cc_out{h}", [NCORES, HD, TPC], f16) for h in range(HPC)]

    if DEBUG:
        dbg_q = nc.dram_tensor("dbg_q", [128, HPC, T], f16, kind="ExternalOutput")
        dbg_k = nc.dram_tensor("dbg_k", [128, T], f16, kind="ExternalOutput")
        dbg_v = nc.dram_tensor("dbg_v", [128, T // 128, HD], f16, kind="ExternalOutput")
        dbg_ag = nc.dram_tensor("dbg_ag", [128, NDC, TPC], f16, kind="ExternalOutput")

    with tile.TileContext(nc) as tc, ExitStack() as ctx, nc.allow_low_precision(
        reason="f16 tiles are full fp32 storage; all accumulation is fp32 PSUM"
    ):
        consts = ctx.enter_context(tc.tile_pool(name="consts", bufs=1))
        xtp = ctx.enter_context(tc.tile_pool(name="xtp", bufs=18))
        qkvp = ctx.enter_context(tc.tile_pool(name="qkvp", bufs=1))
        ropep = ctx.enter_context(tc.tile_pool(name="ropep", bufs=3))
        halfp = ctx.enter_context(tc.tile_pool(name="halfp", bufs=6))
        statp = ctx.enter_context(tc.tile_pool(name="statp", bufs=4))
        sap = ctx.enter_context(tc.tile_pool(name="sap", bufs=6))
        expp = ctx.enter_context(tc.tile_pool(name="expp", bufs=8))
        attp = ctx.enter_context(tc.tile_pool(name="attp", bufs=4))
        wop = ctx.enter_context(tc.tile_pool(name="wop", bufs=6))
        wop2 = ctx.enter_context(tc.tile_pool(name="wop2", bufs=6))
        yp = ctx.enter_context(tc.tile_pool(name="yp", bufs=2))
        yep = ctx.enter_context(tc.tile_pool(name="yep", bufs=16))

        pacc = ctx.enter_context(tc.tile_pool(name="pacc", bufs=2, space="PSUM"))
        pstream = ctx.enter_context(tc.tile_pool(name="pstream", bufs=5, space="PSUM"))
        pmisc = ctx.enter_context(tc.tile_pool(name="pmisc", bufs=1, space="PSUM"))

        # ---- constants ----
        ones_f = consts.tile([128, 1], f32)
        nc.vector.memset(ones_f, 1.0)
        ones = consts.tile([128, 1], f16)
        nc.vector.tensor_copy(out=ones, in_=ones_f)
        ones_k1_f = consts.tile([1, 128], f32)
        nc.vector.memset(ones_k1_f, 1.0)
        ones_k1 = consts.tile([1, 128], f16)
        nc.vector.tensor_copy(out=ones_k1, in_=ones_k1_f)
        ident = consts.tile([128, 128], f16)
        make_identity(nc, ident)
        eps_t = consts.tile([1, 1], f32)
        nc.vector.memset(eps_t, EPS)
        # LUTs duplicated into both partition halves so rope tensor_tensor
        # ops always see matching base partitions
        cos_sb = consts.tile([128, L], f16)
        nc.gpsimd.dma_start(out=cos_sb[0:64, :], in_=lcos[:, :])
        nc.gpsimd.dma_start(out=cos_sb[64:128, :], in_=lcos[:, :])
        sin_sb = consts.tile([128, L], f16)
        nc.gpsimd.dma_start(out=sin_sb[0:64, :], in_=lsin[:, :])
        nc.gpsimd.dma_start(out=sin_sb[64:128, :], in_=lsin[:, :])
        qn_sb = consts.tile([HD, 1], f32)
        nc.gpsimd.dma_start(out=qn_sb, in_=qn[:, :])
        kn_sb = consts.tile([HD, 1], f32)
        nc.gpsimd.dma_start(out=kn_sb, in_=kn[:, :])

        # ---- persistent activations, per batch ----
        qh_t = [
            [
                qkvp.tile([128, L], f16, tag=f"ag{h}", bufs=2, name=f"qh{h}{b}")
                for b in range(B)
            ]
            for h in range(HPC)
        ]
        kh_t = [
            qkvp.tile([128, L], f16, tag=f"kh{b}", name=f"kh{b}") for b in range(B)
        ]
        v_t = [
            qkvp.tile([128, L // 128, HD], f16, tag=f"v{b}", name=f"v{b}")
            for b in range(B)
        ]

        # ---- weights for qkv projection (resident) ----
        w_sb = consts.tile([128, NDC, 512], f16)
        _wr = wqkv.ap().rearrange("(dc p) f -> p dc f", p=128)
        nc.sync.dma_start(out=w_sb[:, 0:4, :], in_=_wr[:, 0:4, :])
        nc.sync.dma_start(out=w_sb[:, 4:NDC, :], in_=_wr[:, 4:NDC, :])

        def proj_tt(b, tt):
            """QKV projection + RoPE + RMSNorm for one 512-token tile.

            Uses only 2 PSUM slots (two fc passes over resident x tiles) so a
            concurrent attention stream can hold the other two."""
            pos0 = tt * TT
            tok0 = b * L + tt * TT
            xts = []
            for dc in range(NDC):
                xt = xtp.tile([128, TT], f16, tag="xt")
                nc.sync.dma_start(
                    out=xt,
                    in_=xT[dc * 128:(dc + 1) * 128, tok0:tok0 + TT],
                )
                xts.append(xt)
            for fc in range(NFC):
                pp = pacc.tile([128, TT], f32, tag="pacc", name=f"pp{fc}")
                for dc in range(NDC):
                    nc.tensor.matmul(
                        pp,
                        w_sb[:, dc, fc * 128:(fc + 1) * 128],
                        xts[dc],
                        start=(dc == 0),
                        stop=(dc == NDC - 1),
                    )
                if True:
                    if fc < 3:
                        # copy psum to sbuf once (frees psum), rope on fp16
                        rsrc = ropep.tile([128, TT], f16, tag="rsrc")
                        nc.scalar.copy(out=rsrc, in_=pp)
                        cs_lo = cos_sb[0:64, pos0:pos0 + TT]
                        cs_hi = cos_sb[64:128, pos0:pos0 + TT]
                        sn_lo = sin_sb[0:64, pos0:pos0 + TT]
                        sn_hi = sin_sb[64:128, pos0:pos0 + TT]
                        x1 = rsrc[0:64, :]
                        x2 = rsrc[64:128, :]
                        t1 = halfp.tile([64, TT], f16, tag="half")
                        t2 = halfp.tile([64, TT], f16, tag="half")
                        t3 = halfp.tile([64, TT], f16, tag="half")
                        t4 = halfp.tile([64, TT], f16, tag="half")
                        roped = ropep.tile([128, TT], f16, tag="roped")
                        nc.vector.tensor_tensor(out=t1, in0=x1, in1=cs_lo, op=mult)
                        nc.vector.tensor_tensor(out=t2, in0=x2, in1=sn_hi, op=mult)
                        nc.vector.tensor_tensor(
                            out=roped[0:64, :], in0=t1, in1=t2, op=sub
                        )
                        nc.vector.tensor_tensor(out=t3, in0=x2, in1=cs_hi, op=mult)
                        nc.vector.tensor_tensor(out=t4, in0=x1, in1=sn_lo, op=mult)
                        nc.vector.tensor_tensor(
                            out=roped[64:128, :], in0=t3, in1=t4, op=add
                        )
                        # sum of squares over HD (partition) via ones-matmul
                        sq = ropep.tile([128, TT], f16, tag="sq")
                        nc.vector.tensor_tensor(out=sq, in0=roped, in1=roped, op=mult)
                        pss = pstream.tile([1, TT], f32, tag="pstream")
                        nc.tensor.matmul(pss, ones, sq, start=True, stop=True)
                        # rstd = exp(-0.5*ln(ss/HD + eps)) -- Ln/Exp share one table
                        lnt = statp.tile([1, TT], f32, tag="stat")
                        nc.scalar.activation(
                            out=lnt, in_=pss, func=Ln, bias=eps_t, scale=1.0 / HD
                        )
                        srd = statp.tile([1, TT], f16, tag="stat")
                        nc.scalar.activation(out=srd, in_=lnt, func=Exp, scale=-0.5)
                        # broadcast rstd over partitions via K=1 matmul
                        pb = pmisc.tile([128, TT], f32, tag="pmisc")
                        nc.tensor.matmul(pb, ones_k1, srd, start=True, stop=True)
                        # final: out = (roped * norm_w) * rstd_bcast
                        w_head = qn_sb if fc < 2 else kn_sb
                        if fc < 2:
                            dst = qh_t[fc][b][:, pos0:pos0 + TT]
                        else:
                            dst = kh_t[b][:, pos0:pos0 + TT]
                        nc.vector.scalar_tensor_tensor(
                            out=dst, in0=roped, scalar=w_head, in1=pb,
                            op0=mult, op1=mult,
                        )
                    else:
                        # v: copy out and transpose to [tok, HD]
                        vt = ropep.tile([128, TT], f16, tag="rsrc")
                        nc.scalar.copy(out=vt, in_=pp)
                        for i in range(TT // 128):
                            pt = pstream.tile([128, 128], f16, tag="pstream")
                            nc.tensor.transpose(
                                pt, vt[:, i * 128:(i + 1) * 128], ident
                            )
                            nc.scalar.copy(out=v_t[b][:, tt * 4 + i, :], in_=pt)

        def att_tiles(jobs):
            """Interleaved attention for a list of (hc, b, tqt) query tiles.

            The softmax denominator is accumulated on the vector engine
            (sacc += exp tile) so the PE only runs 2 matmuls per key chunk;
            one partition-sum matmul per query tile closes it out."""
            NK = L // 128
            state = []
            for hc, b, tqt in jobs:
                qs = qh_t[hc][b][:, tqt * TT:(tqt + 1) * TT]
                po = pacc.tile([128, TT], f32, tag="pacc", name=f"po{hc}{b}{tqt}")
                sacc = [
                    sap.tile([128, TT], f16, tag="sacc", name=f"sa{p}{hc}{b}{tqt}")
                    for p in range(2)
                ]
                state.append((hc, b, tqt, qs, po, sacc))
            for tk in range(NK):
                ets = []
                for hc, b, tqt, qs, po, sacc in state:
                    ps = pstream.tile([128, TT], f32, tag="pstream")
                    nc.tensor.matmul(
                        ps, kh_t[b][:, tk * 128:(tk + 1) * 128], qs,
                        start=True, stop=True,
                    )
                    et = expp.tile([128, TT], f16, tag="expt")
                    nc.scalar.activation(out=et, in_=ps, func=Exp, scale=SCALE)
                    ets.append(et)
                for (hc, b, tqt, qs, po, sacc), et in zip(state, ets):
                    nc.tensor.matmul(
                        po, v_t[b][:, tk, :], et,
                        start=(tk == 0), stop=(tk == NK - 1),
                    )
                    sa = sacc[tk % 2]
                    if tk < 2:
                        nc.vector.tensor_copy(out=sa, in_=et)
                    else:
                        nc.vector.tensor_tensor(out=sa, in0=sa, in1=et, op=add)
            for hc, b, tqt, qs, po, sacc in state:
                # denominator: partition-sum of both fp16 partials, 1/x, bcast
                pd = pmisc.tile([1, TT], f32, tag="pmisc")
                nc.tensor.matmul(pd, ones, sacc[0], start=True, stop=False)
                nc.tensor.matmul(pd, ones, sacc[1], start=False, stop=True)
                rdf = statp.tile([1, TT], f32, tag="stat")
                nc.vector.reciprocal_approx_fast(out=rdf, in_=pd)
                rd = statp.tile([1, TT], f16, tag="stat")
                nc.vector.tensor_copy(out=rd, in_=rdf)
                pb = pmisc.tile([128, TT], f32, tag="pmisc")
                nc.tensor.matmul(pb, ones_k1, rd, start=True, stop=True)
                o_sb = attp.tile([128, TT], f32, tag="att")
                nc.vector.tensor_copy(out=o_sb, in_=po)
                aout = attp.tile([128, TT], f16, tag="att")
                nc.vector.tensor_tensor(out=aout, in0=o_sb, in1=pb, op=mult)
                j = b * NTT + tqt
                nc.sync.dma_start(out=cc_in[hc][j, :, :], in_=aout)

        def att_tile(hc, b, tqt):
            att_tiles([(hc, b, tqt)])

        # ---- phase schedule ----
        for tt in range(NTT):
            proj_tt(0, tt)
        # att(0,0) interleaved with proj(1): 2+2 PSUM slots
        for tqt in range(NTT):
            att_tile(0, 0, tqt)
            proj_tt(1, tqt)
        att_tiles([(0, 1, 0), (0, 1, 1)])
        att_tiles([(0, 1, 2), (0, 1, 3)])
        att_tiles([(1, 0, 0), (1, 0, 1)])
        att_tiles([(1, 0, 2), (1, 0, 3)])

        nc.gpsimd.collective_compute(
            "AllToAll",
            mybir.AluOpType.bypass,
            replica_groups=[list(range(NCORES))],
            ins=[cc_in[0].ap()],
            outs=[cc_out[0].ap()],
        )
        ag0a = qkvp.tile([128, 4, TPC], f16, tag="ag0", bufs=2, name="ag0a")
        ag0b = qkvp.tile([128, 4, TPC], f16, tag="ag0", bufs=2, name="ag0b")
        nc.sync.dma_start(
            out=ag0a,
            in_=cc_out[0].ap()[0:4, :, :].rearrange("j p t -> p j t"),
        )
        nc.sync.dma_start(
            out=ag0b,
            in_=cc_out[0].ap()[4:8, :, :].rearrange("j p t -> p j t"),
        )

        # out-projection is split: the even f-chunks (from ag0) accumulate
        # into SBUF partials during att(1,1) and the second AllToAll; the odd
        # half finishes after ag1 arrives.
        wos_e = {}
        wos_o = {}

        def load_wos(dc, parity, engine):
            store = wos_e if parity == 0 else wos_o
            pool = wop if parity == 0 else wop2
            store[dc] = pool.tile(
                [128, NDC // 2, 128], f16, tag=f"wo{parity}", name=f"wos{parity}_{dc}"
            )
            engine.dma_start(
                out=store[dc],
                in_=wo[:, dc * 128:(dc + 1) * 128].rearrange(
                    "(fc p) m -> p fc m", p=128
                )[:, parity::2, :],
            )

        ye_t = {}

        def even_pass(dc):
            py = pacc.tile([128, TPC], f32, tag="pacc", name="pye")
            for j in range(NDC // 2):
                srct = ag0a if j < 4 else ag0b
                nc.tensor.matmul(
                    py, wos_e[dc][:, j, :], srct[:, j % 4, :],
                    start=(j == 0), stop=(j == 7),
                )
            ye = yep.tile([128, TPC], f16, tag="ye", name=f"ye{dc}")
            nc.vector.tensor_copy(out=ye, in_=py)
            ye_t[dc] = ye

        # att(1,1) first (so the second AllToAll can fire early), then the
        # even half of the out-projection overlaps that AllToAll
        att_tiles([(1, 1, 0), (1, 1, 1)])
        att_tiles([(1, 1, 2), (1, 1, 3)])
        for dc in range(NDC):
            load_wos(dc, 0, nc.sync)
            even_pass(dc)

        nc.gpsimd.collective_compute(
            "AllToAll",
            mybir.AluOpType.bypass,
            replica_groups=[list(range(NCORES))],
            ins=[cc_in[1].ap()],
            outs=[cc_out[1].ap()],
        )
        ag1a = qkvp.tile([128, 4, TPC], f16, tag="ag1", bufs=2, name="ag1a")
        ag1b = qkvp.tile([128, 4, TPC], f16, tag="ag1", bufs=2, name="ag1b")
        nc.sync.dma_start(
            out=ag1a,
            in_=cc_out[1].ap()[0:4, :, :].rearrange("j p t -> p j t"),
        )
        nc.sync.dma_start(
            out=ag1b,
            in_=cc_out[1].ap()[4:8, :, :].rearrange("j p t -> p j t"),
        )

        if DEBUG:
            for h in range(HPC):
                for b in range(B):
                    nc.sync.dma_start(
                        out=dbg_q.ap()[:, h, b * L:(b + 1) * L],
                        in_=qh_t[h][b],
                    )
            for b in range(B):
                nc.sync.dma_start(
                    out=dbg_k.ap()[:, b * L:(b + 1) * L], in_=kh_t[b]
                )
                nc.sync.dma_start(
                    out=dbg_v.ap()[:, b * 16:(b + 1) * 16, :], in_=v_t[b]
                )
            for j in range(NCORES):
                a0 = ag0a if j < 4 else ag0b
                a1 = ag1a if j < 4 else ag1b
                nc.sync.dma_start(out=dbg_ag.ap()[:, 2 * j, :], in_=a0[:, j % 4, :])
                nc.sync.dma_start(out=dbg_ag.ap()[:, 2 * j + 1, :], in_=a1[:, j % 4, :])

        # odd half + combine + store; first slices prefetch during the A2A
        for dc in range(6):
            load_wos(dc, 1, nc.sync)
        for dc in range(NDC):
            if dc not in wos_o:
                load_wos(dc, 1, nc.sync)
            py = pacc.tile([128, TPC], f32, tag="pacc", name="pyo")
            for j in range(NDC // 2):
                srct = ag1a if j < 4 else ag1b
                nc.tensor.matmul(
                    py, wos_o[dc][:, j, :], srct[:, j % 4, :],
                    start=(j == 0), stop=(j == 7),
                )
            yt = yp.tile([128, TPC], f32, tag="y")
            nc.vector.tensor_tensor(out=yt, in0=py, in1=ye_t[dc], op=add)
            nc.sync.dma_start(out=yT[dc * 128:(dc + 1) * 128, :], in_=yt)

    nc.finalize()
    return nc


def kernel(x, wq, wk, wv, wo, qn_w, kn_w):
    from concourse.bass_utils import run_bass_kernel_spmd

    if "nc" not in _CACHE:
        _CACHE["nc"] = _build_nc()
    nc = _CACHE["nc"]

    x = np.asarray(x, dtype=np.float32)
    wq = np.asarray(wq, dtype=np.float32)
    wk = np.asarray(wk, dtype=np.float32)
    wv = np.asarray(wv, dtype=np.float32)
    wo = np.asarray(wo, dtype=np.float32)
    qn_w = np.asarray(qn_w, dtype=np.float32).reshape(HD, 1).copy()
    kn_w = np.asarray(kn_w, dtype=np.float32).reshape(HD, 1).copy()

    xT = np.ascontiguousarray(x.reshape(T, D).T.astype(np.float16))
    wo16 = wo.astype(np.float16)
    cos, sin = _rope_tables()
    cos = cos.astype(np.float16)
    sin = sin.astype(np.float16)

    in_maps = []
    for c in range(NCORES):
        wqkv_c = np.ascontiguousarray(
            np.concatenate(
                [
                    wq[:, c * HPC * HD:(c + 1) * HPC * HD],
                    wk[:, c * HD:(c + 1) * HD],
                    wv[:, c * HD:(c + 1) * HD],
                ],
                axis=1,
            ).astype(np.float16)
        )
        in_maps.append(
            {
                "xT": xT,
                "wqkv": wqkv_c,
                "wo": wo16,
                "lcos": cos,
                "lsin": sin,
                "qn": qn_w,
                "kn": kn_w,
            }
        )

    trace = bool(_CACHE.get("trace"))
    r = run_bass_kernel_spmd(
        nc, in_maps, core_ids=list(range(NCORES)), trace=trace
    )
    _CACHE["last_result"] = r

    y = np.empty((T, D), dtype=np.float32)
    for c in range(NCORES):
        y[c * TPC:(c + 1) * TPC, :] = r.results[c]["yT"].T
    return y.reshape(B, L, D)


# revision 47
# speedup vs baseline: 1.0849x; 1.0148x over previous
"""Trainium2 Bass kernel for GQA attention (B=2, L=2048, D=2048, H=16, KV=8, HD=128).

Sharding: tensor-parallel over heads across 8 cores (2 Q heads + 1 KV head per
core), flash-style attention per core, then two AllToAlls (one per local head)
to redistribute from head-sharding to token-sharding before the output
projection (each core computes 512 full output rows; host concatenates).

All matmuls run as fp32r (full fp32 storage, 1 cycle/row at N>=512).
1/sqrt and 1/x are computed via the ACT Dsqrt table (0.5*x^-0.5) broadcast
through a K=1 matmul with a 2.0 stationary vector.
"""
import math
import numpy as np

B, L, D = 2, 2048, 2048
H, KV, HD = 16, 8, 128
NCORES = 8
T = B * L            # 4096 tokens, b-major
TPC = T // NCORES    # 512 tokens per core after A2A
HPC = H // NCORES    # 2 local query heads
EPS = 1e-5
ROPE_BASE = 10000.0
SCALE = HD ** -0.5

TT = 512             # token tile (free dim)
NTT = L // TT        # 4 token tiles per batch
NDC = D // 128       # 16 contraction chunks
NFC = 4              # output col chunks of 128 in qkv proj (2 q heads + k + v)

_CACHE = {}
DEBUG = False


def _rope_tables():
    """cos/sin LUTs [64, L] computed exactly like the jax reference (f32, cpu)."""
    import jax
    import jax.numpy as jnp

    cpu = jax.devices("cpu")[0]
    with jax.default_device(cpu):
        base = ROPE_BASE * 1.0 ** (HD / (HD - 2))
        freqs = base ** (jnp.arange(0, HD, 2, dtype=jnp.float32) / HD)   # [64]
        pos = jnp.arange(L, dtype=jnp.float32)                           # [L]
        angles = pos[:, None] * freqs[None, :]                           # [L, 64]
        cos = np.asarray(jnp.cos(angles), dtype=np.float32).T.copy()     # [64, L]
        sin = np.asarray(jnp.sin(angles), dtype=np.float32).T.copy()
    return cos, sin


def _build_nc():
    import concourse.bass as bass
    import concourse.tile as tile
    from concourse.tile import add_dep_helper
    import concourse.mybir as mybir
    from concourse import bacc
    from concourse.masks import make_identity
    from contextlib import ExitStack

    f32 = mybir.dt.float32
    f16 = mybir.dt.float16
    Exp = mybir.ActivationFunctionType.Exp
    Ln = mybir.ActivationFunctionType.Ln
    mult = mybir.AluOpType.mult
    add = mybir.AluOpType.add
    sub = mybir.AluOpType.subtract

    from concourse import bacc as _bacc_mod
    from concourse import hw_specs as _hw

    if not getattr(_bacc_mod, "_act_table_patch", False):
        _orig_get = _bacc_mod.get_activation_tables

        def _patched_get(arch):
            t = _orig_get(arch)
            exp = mybir.ActivationFunctionType.Exp
            ln = mybir.ActivationFunctionType.Ln
            for name, funcs in t.items():
                if name != "natural_log_exp_and_others":
                    funcs.discard(exp)
                    funcs.discard(ln)
            return t

        _bacc_mod.get_activation_tables = _patched_get
        _bacc_mod._act_table_patch = True

    nc = bacc.Bacc(num_devices=NCORES)

    xT = nc.dram_tensor("xT", [D, T], f16, kind="ExternalInput")
    wqkv = nc.dram_tensor("wqkv", [D, 512], f16, kind="ExternalInput")
    wo = nc.dram_tensor("wo", [D, D], f16, kind="ExternalInput")
    lcos = nc.dram_tensor("lcos", [64, L], f16, kind="ExternalInput")
    lsin = nc.dram_tensor("lsin", [64, L], f16, kind="ExternalInput")
    qn = nc.dram_tensor("qn", [HD, 1], f32, kind="ExternalInput")
    kn = nc.dram_tensor("kn", [HD, 1], f32, kind="ExternalInput")
    yT = nc.dram_tensor("yT", [D, TPC], f32, kind="ExternalOutput")

    # A2A bounce buffers, one pair per local head chunk
    cc_in = [nc.dram_tensor(f"cc_in{h}", [NCORES, HD, TPC], f16) for h in range(HPC)]
    cc_out = [nc.dram_tensor(f"cc_out{h}", [NCORES, HD, TPC], f16) for h in range(HPC)]

    if DEBUG:
        dbg_q = nc.dram_tensor("dbg_q", [128, HPC, T], f16, kind="ExternalOutput")
        dbg_k = nc.dram_tensor("dbg_k", [128, T], f16, kind="ExternalOutput")
        dbg_v = nc.dram_tensor("dbg_v", [128, T // 128, HD], f16, kind="ExternalOutput")
        dbg_ag = nc.dram_tensor("dbg_ag", [128, NDC, TPC], f16, kind="ExternalOutput")

    with tile.TileContext(nc) as tc, ExitStack() as ctx, nc.allow_low_precision(
        reason="f16 tiles are full fp32 storage; all accumulation is fp32 PSUM"
    ):
        consts = ctx.enter_context(tc.tile_pool(name="consts", bufs=1))
        xtp = ctx.enter_context(tc.tile_pool(name="xtp", bufs=18))
        qkvp = ctx.enter_context(tc.tile_pool(name="qkvp", bufs=1))
        ropep = ctx.enter_context(tc.tile_pool(name="ropep", bufs=3))
        halfp = ctx.enter_context(tc.tile_pool(name="halfp", bufs=6))
        statp = ctx.enter_context(tc.tile_pool(name="statp", bufs=4))
        sap = ctx.enter_context(tc.tile_pool(name="sap", bufs=6))
        expp = ctx.enter_context(tc.tile_pool(name="expp", bufs=8))
        attp = ctx.enter_context(tc.tile_pool(name="attp", bufs=4))
        wop = ctx.enter_context(tc.tile_pool(name="wop", bufs=6))
        wop2 = ctx.enter_context(tc.tile_pool(name="wop2", bufs=6))
        yp = ctx.enter_context(tc.tile_pool(name="yp", bufs=2))
        yep = ctx.enter_context(tc.tile_pool(name="yep", bufs=16))

        pacc = ctx.enter_context(tc.tile_pool(name="pacc", bufs=2, space="PSUM"))
        pstream = ctx.enter_context(tc.tile_pool(name="pstream", bufs=5, space="PSUM"))
        pmisc = ctx.enter_context(tc.tile_pool(name="pmisc", bufs=1, space="PSUM"))

        # ---- constants ----
        ones_f = consts.tile([128, 1], f32)
        nc.vector.memset(ones_f, 1.0)
        ones = consts.tile([128, 1], f16)
        nc.vector.tensor_copy(out=ones, in_=ones_f)
        ones_k1_f = consts.tile([1, 128], f32)
        nc.vector.memset(ones_k1_f, 1.0)
        ones_k1 = consts.tile([1, 128], f16)
        nc.vector.tensor_copy(out=ones_k1, in_=ones_k1_f)
        ident = consts.tile([128, 128], f16)
        make_identity(nc, ident)
        eps_t = consts.tile([1, 1], f32)
        nc.vector.memset(eps_t, EPS)
        # LUTs duplicated into both partition halves so rope tensor_tensor
        # ops always see matching base partitions
        cos_sb = consts.tile([128, L], f16)
        nc.gpsimd.dma_start(out=cos_sb[0:64, :], in_=lcos[:, :])
        nc.gpsimd.dma_start(out=cos_sb[64:128, :], in_=lcos[:, :])
        sin_sb = consts.tile([128, L], f16)
        nc.gpsimd.dma_start(out=sin_sb[0:64, :], in_=lsin[:, :])
        nc.gpsimd.dma_start(out=sin_sb[64:128, :], in_=lsin[:, :])
        qn_sb = consts.tile([HD, 1], f32)
        nc.gpsimd.dma_start(out=qn_sb, in_=qn[:, :])
        kn_sb = consts.tile([HD, 1], f32)
        nc.gpsimd.dma_start(out=kn_sb, in_=kn[:, :])

        # ---- persistent activations, per batch ----
        qh_t = [
            [
                qkvp.tile([128, L], f16, tag=f"ag{h}", bufs=2, name=f"qh{h}{b}")
                for b in range(B)
            ]
            for h in range(HPC)
        ]
        kh_t = [
            qkvp.tile([128, L], f16, tag=f"kh{b}", name=f"kh{b}") for b in range(B)
        ]
        v_t = [
            qkvp.tile([128, L // 128, HD], f16, tag=f"v{b}", name=f"v{b}")
            for b in range(B)
        ]

        # ---- weights for qkv projection (resident) ----
        w_sb = consts.tile([128, NDC, 512], f16)
        _wr = wqkv.ap().rearrange("(dc p) f -> p dc f", p=128)
        nc.sync.dma_start(out=w_sb[:, 0:4, :], in_=_wr[:, 0:4, :])
        nc.sync.dma_start(out=w_sb[:, 4:NDC, :], in_=_wr[:, 4:NDC, :])

        def proj_tt(b, tt):
            """QKV projection + RoPE + RMSNorm for one 512-token tile.

            Uses only 2 PSUM slots (two fc passes over resident x tiles) so a
            concurrent attention stream can hold the other two."""
            pos0 = tt * TT
            tok0 = b * L + tt * TT
            xts = []
            for dc in range(NDC):
                xt = xtp.tile([128, TT], f16, tag="xt")
                nc.sync.dma_start(
                    out=xt,
                    in_=xT[dc * 128:(dc + 1) * 128, tok0:tok0 + TT],
                )
                xts.append(xt)
            for fc in range(NFC):
                pp = pacc.tile([128, TT], f32, tag="pacc", name=f"pp{fc}")
                for dc in range(NDC):
                    nc.tensor.matmul(
                        pp,
                        w_sb[:, dc, fc * 128:(fc + 1) * 128],
                        xts[dc],
                        start=(dc == 0),
                        stop=(dc == NDC - 1),
                    )
                if True:
                    if fc < 3:
                        # copy psum to sbuf once (frees psum), rope on fp16
                        rsrc = ropep.tile([128, TT], f16, tag="rsrc")
                        nc.vector.tensor_copy(out=rsrc, in_=pp)
                        cs_lo = cos_sb[0:64, pos0:pos0 + TT]
                        cs_hi = cos_sb[64:128, pos0:pos0 + TT]
                        sn_lo = sin_sb[0:64, pos0:pos0 + TT]
                        sn_hi = sin_sb[64:128, pos0:pos0 + TT]
                        x1 = rsrc[0:64, :]
                        x2 = rsrc[64:128, :]
                        t1 = halfp.tile([64, TT], f16, tag="half")
                        t2 = halfp.tile([64, TT], f16, tag="half")
                        t3 = halfp.tile([64, TT], f16, tag="half")
                        t4 = halfp.tile([64, TT], f16, tag="half")
                        roped = ropep.tile([128, TT], f16, tag="roped")
                        nc.vector.tensor_tensor(out=t1, in0=x1, in1=cs_lo, op=mult)
                        nc.vector.tensor_tensor(out=t2, in0=x2, in1=sn_hi, op=mult)
                        nc.vector.tensor_tensor(
                            out=roped[0:64, :], in0=t1, in1=t2, op=sub
                        )
                        nc.vector.tensor_tensor(out=t3, in0=x2, in1=cs_hi, op=mult)
                        nc.vector.tensor_tensor(out=t4, in0=x1, in1=sn_lo, op=mult)
                        nc.vector.tensor_tensor(
                            out=roped[64:128, :], in0=t3, in1=t4, op=add
                        )
                        # sum of squares over HD (partition) via ones-matmul
                        sq = ropep.tile([128, TT], f16, tag="sq")
                        nc.vector.tensor_tensor(out=sq, in0=roped, in1=roped, op=mult)
                        pss = pstream.tile([1, TT], f32, tag="pstream")
                        nc.tensor.matmul(pss, ones, sq, start=True, stop=True)
                        # rstd = exp(-0.5*ln(ss/HD + eps)) -- Ln/Exp share one table
                        lnt = statp.tile([1, TT], f32, tag="stat")
                        nc.scalar.activation(
                            out=lnt, in_=pss, func=Ln, bias=eps_t, scale=1.0 / HD
                        )
                        srd = statp.tile([1, TT], f16, tag="stat")
                        nc.scalar.activation(out=srd, in_=lnt, func=Exp, scale=-0.5)
                        # broadcast rstd over partitions via K=1 matmul
                        pb = pmisc.tile([128, TT], f32, tag="pmisc")
                        nc.tensor.matmul(pb, ones_k1, srd, start=True, stop=True)
                        # final: out = (roped * norm_w) * rstd_bcast
                        w_head = qn_sb if fc < 2 else kn_sb
                        if fc < 2:
                            dst = qh_t[fc][b][:, pos0:pos0 + TT]
                        else:
                            dst = kh_t[b][:, pos0:pos0 + TT]
                        nc.vector.scalar_tensor_tensor(
                            out=dst, in0=roped, scalar=w_head, in1=pb,
                            op0=mult, op1=mult,
                        )
                    else:
                        # v: copy out and transpose to [tok, HD]
                        vt = ropep.tile([128, TT], f16, tag="rsrc")
                        nc.vector.tensor_copy(out=vt, in_=pp)
                        for i in range(TT // 128):
                            pt = pstream.tile([128, 128], f16, tag="pstream")
                            nc.tensor.transpose(
                                pt, vt[:, i * 128:(i + 1) * 128], ident
                            )
                            nc.vector.tensor_copy(out=v_t[b][:, tt * 4 + i, :], in_=pt)

        def att_tiles(jobs):
            """Interleaved attention for a list of (hc, b, tqt) query tiles.

            The softmax denominator is accumulated on the vector engine
            (sacc += exp tile) so the PE only runs 2 matmuls per key chunk;
            one partition-sum matmul per query tile closes it out."""
            NK = L // 128
            state = []
            for hc, b, tqt in jobs:
                qs = qh_t[hc][b][:, tqt * TT:(tqt + 1) * TT]
                po = pacc.tile([128, TT], f32, tag="pacc", name=f"po{hc}{b}{tqt}")
                sacc = [
                    sap.tile([128, TT], f16, tag="sacc", name=f"sa{p}{hc}{b}{tqt}")
                    for p in range(2)
                ]
                state.append((hc, b, tqt, qs, po, sacc))
            for tk in range(NK):
                ets = []
                for hc, b, tqt, qs, po, sacc in state:
                    ps = pstream.tile([128, TT], f32, tag="pstream")
                    nc.tensor.matmul(
                        ps, kh_t[b][:, tk * 128:(tk + 1) * 128], qs,
                        start=True, stop=True,
                    )
                    et = expp.tile([128, TT], f16, tag="expt")
                    nc.scalar.activation(out=et, in_=ps, func=Exp, scale=SCALE)
                    ets.append(et)
                for (hc, b, tqt, qs, po, sacc), et in zip(state, ets):
                    nc.tensor.matmul(
                        po, v_t[b][:, tk, :], et,
                        start=(tk == 0), stop=(tk == NK - 1),
                    )
                    sa = sacc[tk % 2]
                    if tk < 2:
                        nc.vector.tensor_copy(out=sa, in_=et)
                    else:
                        nc.vector.tensor_tensor(out=sa, in0=sa, in1=et, op=add)
            for hc, b, tqt, qs, po, sacc in state:
                # denominator: partition-sum of both fp16 partials, 1/x, bcast
                pd = pmisc.tile([1, TT], f32, tag="pmisc")
                nc.tensor.matmul(pd, ones, sacc[0], start=True, stop=False)
                nc.tensor.matmul(pd, ones, sacc[1], start=False, stop=True)
                rdf = statp.tile([1, TT], f32, tag="stat")
                nc.vector.reciprocal_approx_fast(out=rdf, in_=pd)
                rd = statp.tile([1, TT], f16, tag="stat")
                nc.vector.tensor_copy(out=rd, in_=rdf)
                pb = pmisc.tile([128, TT], f32, tag="pmisc")
                nc.tensor.matmul(pb, ones_k1, rd, start=True, stop=True)
                o_sb = attp.tile([128, TT], f32, tag="att")
                nc.vector.tensor_copy(out=o_sb, in_=po)
                aout = attp.tile([128, TT], f16, tag="att")
                nc.vector.tensor_tensor(out=aout, in0=o_sb, in1=pb, op=mult)
                j = b * NTT + tqt
                nc.sync.dma_start(out=cc_in[hc][j, :, :], in_=aout)

        def att_tile(hc, b, tqt):
            att_tiles([(hc, b, tqt)])

        # ---- phase schedule ----
        for tt in range(NTT):
            proj_tt(0, tt)
        # att(0,0) interleaved with proj(1): 2+2 PSUM slots
        for tqt in range(NTT):
            att_tile(0, 0, tqt)
            proj_tt(1, tqt)
        att_tiles([(0, 1, 0), (0, 1, 1)])
        att_tiles([(0, 1, 2), (0, 1, 3)])
        att_tiles([(1, 0, 0), (1, 0, 1)])
        att_tiles([(1, 0, 2), (1, 0, 3)])

        nc.gpsimd.collective_compute(
            "AllToAll",
            mybir.AluOpType.bypass,
            replica_groups=[list(range(NCORES))],
            ins=[cc_in[0].ap()],
            outs=[cc_out[0].ap()],
        )
        ag0a = qkvp.tile([128, 4, TPC], f16, tag="ag0", bufs=2, name="ag0a")
        ag0b = qkvp.tile([128, 4, TPC], f16, tag="ag0", bufs=2, name="ag0b")
        nc.sync.dma_start(
            out=ag0a,
            in_=cc_out[0].ap()[0:4, :, :].rearrange("j p t -> p j t"),
        )
        nc.sync.dma_start(
            out=ag0b,
            in_=cc_out[0].ap()[4:8, :, :].rearrange("j p t -> p j t"),
        )

        # out-projection is split: the even f-chunks (from ag0) accumulate
        # into SBUF partials during att(1,1) and the second AllToAll; the odd
        # half finishes after ag1 arrives.
        wos_e = {}
        wos_o = {}

        def load_wos(dc, parity, engine):
            store = wos_e if parity == 0 else wos_o
            pool = wop if parity == 0 else wop2
            store[dc] = pool.tile(
                [128, NDC // 2, 128], f16, tag=f"wo{parity}", name=f"wos{parity}_{dc}"
            )
            engine.dma_start(
                out=store[dc],
                in_=wo[:, dc * 128:(dc + 1) * 128].rearrange(
                    "(fc p) m -> p fc m", p=128
                )[:, parity::2, :],
            )

        ye_t = {}

        def even_pass(dc):
            py = pacc.tile([128, TPC], f32, tag="pacc", name="pye")
            for j in range(NDC // 2):
                srct = ag0a if j < 4 else ag0b
                nc.tensor.matmul(
                    py, wos_e[dc][:, j, :], srct[:, j % 4, :],
                    start=(j == 0), stop=(j == 7),
                )
            ye = yep.tile([128, TPC], f16, tag="ye", name=f"ye{dc}")
            nc.vector.tensor_copy(out=ye, in_=py)
            ye_t[dc] = ye

        # att(1,1) first (so the second AllToAll can fire early), then the
        # even half of the out-projection overlaps that AllToAll
        att_tiles([(1, 1, 0), (1, 1, 1)])
        att_tiles([(1, 1, 2), (1, 1, 3)])
        for dc in range(NDC):
            load_wos(dc, 0, nc.sync)
            even_pass(dc)

        nc.gpsimd.collective_compute(
            "AllToAll",
            mybir.AluOpType.bypass,
            replica_groups=[list(range(NCORES))],
            ins=[cc_in[1].ap()],
            outs=[cc_out[1].ap()],
        )
        ag1a = qkvp.tile([128, 4, TPC], f16, tag="ag1", bufs=2, name="ag1a")
        ag1b = qkvp.tile([128, 4, TPC], f16, tag="ag1", bufs=2, name="ag1b")
        nc.sync.dma_start(
            out=ag1a,
            in_=cc_out[1].ap()[0:4, :, :].rearrange("j p t -> p j t"),
        )
        nc.sync.dma_start(
            out=ag1b,
            in_=cc_out[1].ap()[4:8, :, :].rearrange("j p t -> p j t"),
        )

        if DEBUG:
            for h in range(HPC):
                for b in range(B):
                    nc.sync.dma_start(
                        out=dbg_q.ap()[:, h, b * L:(b + 1) * L],
                        in_=qh_t[h][b],
                    )
            for b in range(B):
                nc.sync.dma_start(
                    out=dbg_k.ap()[:, b * L:(b + 1) * L], in_=kh_t[b]
                )
                nc.sync.dma_start(
                    out=dbg_v.ap()[:, b * 16:(b + 1) * 16, :], in_=v_t[b]
                )
            for j in range(NCORES):
                a0 = ag0a if j < 4 else ag0b
                a1 = ag1a if j < 4 else ag1b
                nc.sync.dma_start(out=dbg_ag.ap()[:, 2 * j, :], in_=a0[:, j % 4, :])
                nc.sync.dma_start(out=dbg_ag.ap()[:, 2 * j + 1, :], in_=a1[:, j % 4, :])

        # odd half + combine + store
        for dc in range(NDC):
            load_wos(dc, 1, nc.sync)
            py = pacc.tile([128, TPC], f32, tag="pacc", name="pyo")
            for j in range(NDC // 2):
                srct = ag1a if j < 4 else ag1b
                nc.tensor.matmul(
                    py, wos_o[dc][:, j, :], srct[:, j % 4, :],
                    start=(j == 0), stop=(j == 7),
                )
            yt = yp.tile([128, TPC], f32, tag="y")
            nc.vector.tensor_tensor(out=yt, in0=py, in1=ye_t[dc], op=add)
            nc.sync.dma_start(out=yT[dc * 128:(dc + 1) * 128, :], in_=yt)

    nc.finalize()
    return nc


def kernel(x, wq, wk, wv, wo, qn_w, kn_w):
    from concourse.bass_utils import run_bass_kernel_spmd

    if "nc" not in _CACHE:
        _CACHE["nc"] = _build_nc()
    nc = _CACHE["nc"]

    x = np.asarray(x, dtype=np.float32)
    wq = np.asarray(wq, dtype=np.float32)
    wk = np.asarray(wk, dtype=np.float32)
    wv = np.asarray(wv, dtype=np.float32)
    wo = np.asarray(wo, dtype=np.float32)
    qn_w = np.asarray(qn_w, dtype=np.float32).reshape(HD, 1).copy()
    kn_w = np.asarray(kn_w, dtype=np.float32).reshape(HD, 1).copy()

    xT = np.ascontiguousarray(x.reshape(T, D).T.astype(np.float16))
    wo16 = wo.astype(np.float16)
    cos, sin = _rope_tables()
    cos = cos.astype(np.float16)
    sin = sin.astype(np.float16)

    in_maps = []
    for c in range(NCORES):
        wqkv_c = np.ascontiguousarray(
            np.concatenate(
                [
                    wq[:, c * HPC * HD:(c + 1) * HPC * HD],
                    wk[:, c * HD:(c + 1) * HD],
                    wv[:, c * HD:(c + 1) * HD],
                ],
                axis=1,
            ).astype(np.float16)
        )
        in_maps.append(
            {
                "xT": xT,
                "wqkv": wqkv_c,
                "wo": wo16,
                "lcos": cos,
                "lsin": sin,
                "qn": qn_w,
                "kn": kn_w,
            }
        )

    trace = bool(_CACHE.get("trace"))
    r = run_bass_kernel_spmd(
        nc, in_maps, core_ids=list(range(NCORES)), trace=trace
    )
    _CACHE["last_result"] = r

    y = np.empty((T, D), dtype=np.float32)
    for c in range(NCORES):
        y[c * TPC:(c + 1) * TPC, :] = r.results[c]["yT"].T
    return y.reshape(B, L, D)
